# revision 2
# baseline (speedup 1.0000x reference)
"""GCNConv Trainium2 kernel: sigmoid(segment_sum(edge_val * (X@W)[edge_col], edge_row) + bias).

Uses the reassociation A@(XW) = (A@X)W:
  - Shard destination rows across 8 cores (12500 rows each); edges partitioned by
    dest row (edge_row is sorted). One NEFF runs SPMD on all 8 cores; per-core
    behavior differs only through input data (gather indices + one-hot metadata).
  - For each 128-row output window and each of 4 node-quadrants (int16 index
    range), dma_gather the referenced raw X rows (f32, 512B descriptors).
  - Per 128-edge slot, build a val-scaled one-hot on the DVE with a single
    tensor_scalar: OH[p,j] = (iota[j] == row_local[p]) * val[p], then
    scatter-add via PE matmul lhsT=gathered-X, rhs=OH accumulating the
    TRANSPOSED window aggregate [feat, rows] in PSUM.
  - Window epilogue: copy PSUM->SBUF, z = W^T-free matmul (lhsT=W), then one
    ACT op computes sigmoid(z + bias) with bias on the partition axis, DMA out
    to a transposed [128, rows] output; host transposes once at the end.
"""
import sys

sys.path.insert(0, "/opt/trn_rl_repo")

import numpy as np

import concourse.mybir as mybir
import concourse.tile as tile
from concourse import bacc
import concourse.bass_utils as bass_utils

# Problem constants (contest contract)
N_NODES = 100000
F = 128
NCORES = 8
ROWS_PER_CORE = N_NODES // NCORES          # 12500
P = 128
N_WIN = -(-ROWS_PER_CORE // P)             # 98 windows/core (last covers 84 rows)
N_QUAD = 4
QUAD = N_NODES // N_QUAD                   # 25000 (< int16 max)
GBUFS = 16                                 # gather tiles in flight
NQUEUES = 4

_cache = {}


def _build_program(C_wq):
    """Build + compile the SPMD program. C_wq: [N_WIN][N_QUAD] chunks per cell
    (identical across cores). Returns compiled Bacc."""
    dt = mybir.dt
    C_wq = [list(map(int, row)) for row in C_wq]
    S = sum(sum(row) for row in C_wq)          # total 128-edge slots
    CMAX = max(max(row) for row in C_wq)

    nc = bacc.Bacc("TRN2", target_bir_lowering=False, debug=False,
                   enable_asserts=False, num_devices=NCORES,
                   num_swdge_queues=NQUEUES)

    x_d = nc.dram_tensor("x", [N_NODES, F], dt.float32, kind="ExternalInput")
    w_d = nc.dram_tensor("w", [F, F], dt.float32, kind="ExternalInput")
    bias_d = nc.dram_tensor("bias", [F, 1], dt.float32, kind="ExternalInput")
    iota_d = nc.dram_tensor("iota", [P, P], dt.float32, kind="ExternalInput")
    gidx_d = nc.dram_tensor("gidx", [P, S * 8], dt.int16, kind="ExternalInput")
    rowl_d = nc.dram_tensor("rowl", [P, S], dt.float32, kind="ExternalInput")
    val_d = nc.dram_tensor("val", [P, S], dt.float32, kind="ExternalInput")
    yt_d = nc.dram_tensor("yt", [F, N_WIN * P], dt.float32, kind="ExternalOutput")

    with tile.TileContext(nc) as tc:
        with (
            tc.tile_pool(name="cst", bufs=1) as cst,
            tc.tile_pool(name="sbg", bufs=GBUFS) as sbg,
            tc.tile_pool(name="sbo", bufs=8) as sbo,
            tc.tile_pool(name="sby", bufs=4) as sby,
            tc.tile_pool(name="psw", bufs=4, space="PSUM") as psw,
            tc.tile_pool(name="psz", bufs=3, space="PSUM") as psz,
        ):
            iota_t = cst.tile([P, P], dt.float32)
            nc.sync.dma_start(iota_t[:], iota_d[:])
            w_t = cst.tile([F, F], dt.float32)
            nc.sync.dma_start(w_t[:], w_d[:])
            bias_t = cst.tile([F, 1], dt.float32)
            nc.sync.dma_start(bias_t[:], bias_d[:])
            gidx_t = cst.tile([P, S * 8], dt.int16)
            nc.sync.dma_start(gidx_t[:], gidx_d[:])
            rowl_t = cst.tile([P, S], dt.float32)
            nc.sync.dma_start(rowl_t[:], rowl_d[:])
            val_t = cst.tile([P, S], dt.float32)
            nc.sync.dma_start(val_t[:], val_d[:])

            s = 0       # global slot cursor; gather (w,q) covers slots [s, s+C)
            qn = 0
            for w in range(N_WIN):
                g = []
                w_slot0 = s
                for q in range(N_QUAD):
                    C = C_wq[w][q]
                    gq = sbg.tile([P, CMAX, F], dt.float32, tag="g")
                    nc.gpsimd.dma_gather(
                        out_ap=gq[:, :C, :],
                        in_ap=x_d[q * QUAD : (q + 1) * QUAD, :],
                        idxs_ap=gidx_t[:, s * 8 : (s + C) * 8],
                        num_idxs=C * P,
                        num_idxs_reg=C * P,
                        elem_size=F,
                        queue_num=qn % NQUEUES,
                    )
                    qn += 1
                    g.append(gq)
                    s += C
                nslots = s - w_slot0
                pw = psw.tile([F, P], dt.float32, tag="pw")
                j = 0
                for q in range(N_QUAD):
                    for k in range(C_wq[w][q]):
                        sl = w_slot0 + j
                        oh = sbo.tile([P, P], dt.float32, tag="oh")
                        nc.vector.tensor_scalar(
                            out=oh[:], in0=iota_t[:],
                            scalar1=rowl_t[:, sl : sl + 1],
                            scalar2=val_t[:, sl : sl + 1],
                            op0=mybir.AluOpType.is_equal,
                            op1=mybir.AluOpType.mult,
                        )
                        nc.tensor.matmul(
                            pw[:], lhsT=g[q][:, k, :], rhs=oh[:],
                            start=(j == 0), stop=(j == nslots - 1),
                        )
                        j += 1
                at = sby.tile([F, P], dt.float32, tag="at")
                nc.scalar.activation(at[:], pw[:], mybir.ActivationFunctionType.Copy)
                z = psz.tile([F, P], dt.float32, tag="z")
                nc.tensor.matmul(z[:], lhsT=w_t[:], rhs=at[:], start=True, stop=True)
                ys = sby.tile([F, P], dt.float32, tag="ys")
                nc.scalar.activation(ys[:], z[:],
                                     mybir.ActivationFunctionType.Sigmoid,
                                     bias=bias_t[:, 0:1])
                nc.sync.dma_start(yt_d[:, w * P : (w + 1) * P], ys[:])

    nc.compile()
    return nc


def _preprocess(X, edge_row, edge_col, edge_val, weight, bias):
    edge_row = np.asarray(edge_row, dtype=np.int64)
    edge_col = np.asarray(edge_col, dtype=np.int64)
    edge_val = np.asarray(edge_val, dtype=np.float32)

    if not np.all(edge_row[:-1] <= edge_row[1:]):
        o = np.argsort(edge_row, kind="stable")
        edge_row, edge_col, edge_val = edge_row[o], edge_col[o], edge_val[o]

    core = edge_row // ROWS_PER_CORE
    rl = edge_row % ROWS_PER_CORE
    wcore = rl // P
    rowl_v = (rl - wcore * P).astype(np.float32)
    quad = edge_col // QUAD
    colq = (edge_col % QUAD).astype(np.int16)

    key = ((core * N_WIN + wcore) * N_QUAD + quad).astype(np.int64)
    order = np.argsort(key, kind="stable")
    key_s = key[order]
    ncells = NCORES * N_WIN * N_QUAD
    counts = np.bincount(key_s, minlength=ncells).reshape(NCORES, N_WIN, N_QUAD)

    # chunks per (window, quadrant): max over cores so the program is SPMD-safe
    C_wq = -(-counts.max(axis=0) // P)           # [N_WIN, N_QUAD]
    C_wq = np.maximum(C_wq, 1)
    cell_cap = C_wq * P                          # idx per cell
    # flat slot space: cells in (w, q) order, each C_wq[w,q]*128 wide
    cell_off = np.zeros((N_WIN, N_QUAD), dtype=np.int64)
    flat = cell_cap.reshape(-1).cumsum()
    cell_off.reshape(-1)[1:] = flat[:-1]
    S = int(flat[-1]) // P

    starts = np.zeros(ncells, dtype=np.int64)
    starts[1:] = np.cumsum(counts.reshape(-1))[:-1]
    rank = np.arange(len(key_s), dtype=np.int64) - starts[key_s]

    core_s = core[order]
    w_s = wcore[order]
    q_s = quad[order]
    pos = cell_off[w_s, q_s] + rank              # flat position within core

    rowl_flat = np.full((NCORES, S * P), -1.0, dtype=np.float32)
    val_flat = np.zeros((NCORES, S * P), dtype=np.float32)
    gidx_flat = np.zeros((NCORES, S * P), dtype=np.int16)
    rowl_flat[core_s, pos] = rowl_v[order]
    val_flat[core_s, pos] = edge_val[order]
    gidx_flat[core_s, pos] = colq[order]

    rowl_m = np.ascontiguousarray(rowl_flat.reshape(NCORES, S, P).transpose(0, 2, 1))
    val_m = np.ascontiguousarray(val_flat.reshape(NCORES, S, P).transpose(0, 2, 1))
    # wrapped gather-index layout: within each slot-region, idx j -> partition
    # j%16 (replicated across the 8 16-partition groups), free column j//16.
    # Gathers cover whole cells = contiguous slot ranges, and slot size (128)
    # is a multiple of the 16-wrap, so wrapping slot-by-slot is equivalent.
    gw = gidx_flat.reshape(NCORES, S, 8, 16).transpose(0, 1, 3, 2)  # [NC,S,16,8]
    gw = np.tile(gw, (1, 1, 8, 1))                                  # [NC,S,128,8]
    gidx_m = np.ascontiguousarray(gw.transpose(0, 2, 1, 3).reshape(NCORES, P, S * 8))

    iota = np.tile(np.arange(P, dtype=np.float32)[None, :], (P, 1))
    shared = {
        "x": np.ascontiguousarray(np.asarray(X, dtype=np.float32)),
        "w": np.ascontiguousarray(np.asarray(weight, dtype=np.float32)),
        "bias": np.ascontiguousarray(np.asarray(bias, dtype=np.float32)[:, None]),
        "iota": iota,
    }
    per_core = [
        {"gidx": np.ascontiguousarray(gidx_m[c]),
         "rowl": np.ascontiguousarray(rowl_m[c]),
         "val": np.ascontiguousarray(val_m[c])}
        for c in range(NCORES)
    ]
    return C_wq, shared, per_core


def kernel(X, edge_row, edge_col, edge_val, weight, bias):
    C_wq, shared, per_core = _preprocess(X, edge_row, edge_col, edge_val,
                                         weight, bias)
    ckey = C_wq.tobytes()
    if ckey not in _cache:
        _cache[ckey] = _build_program(C_wq)
    nc = _cache[ckey]

    in_maps = [dict(shared, **per_core[c]) for c in range(NCORES)]
    res = bass_utils.run_bass_kernel_spmd(nc, in_maps, core_ids=list(range(NCORES)))

    out = np.empty((N_NODES, F), dtype=np.float32)
    for c in range(NCORES):
        out[c * ROWS_PER_CORE : (c + 1) * ROWS_PER_CORE] = \
            res.results[c]["yt"].T[:ROWS_PER_CORE]
    return out


# revision 8
# speedup vs baseline: 2.0826x; 2.0826x over previous
"""GCNConv Trainium2 kernel: sigmoid(segment_sum(edge_val * (X@W)[edge_col], edge_row) + bias).

Uses the reassociation A@(XW) = (A@X)W:
  - Shard destination rows across 8 cores (12500 rows each); edges partitioned by
    dest row (edge_row is sorted). One NEFF runs SPMD on all 8 cores; per-core
    behavior differs only through input data (gather indices + one-hot metadata).
  - For each 128-row output window and each of 4 node-quadrants (int16 index
    range), dma_gather the referenced raw X rows (f32, 512B descriptors).
  - Per 128-edge slot, build a val-scaled one-hot on the DVE with a single
    tensor_scalar: OH[p,j] = (iota[j] == row_local[p]) * val[p], then
    scatter-add via PE matmul lhsT=gathered-X, rhs=OH accumulating the
    TRANSPOSED window aggregate [feat, rows] in PSUM.
  - Window epilogue: copy PSUM->SBUF, z = W^T-free matmul (lhsT=W), then one
    ACT op computes sigmoid(z + bias) with bias on the partition axis, DMA out
    to a transposed [128, rows] output; host transposes once at the end.
"""
import sys

sys.path.insert(0, "/opt/trn_rl_repo")

import numpy as np

import concourse.mybir as mybir
import concourse.tile as tile
from concourse import bacc
import concourse.bass_utils as bass_utils

# Problem constants (contest contract)
N_NODES = 100000
F = 128
NCORES = 8
ROWS_PER_CORE = N_NODES // NCORES          # 12500
P = 128
N_WIN = -(-ROWS_PER_CORE // P)             # 98 windows/core (last covers 84 rows)
N_QUAD = 4
QUAD = N_NODES // N_QUAD                   # 25000 (< int16 max)
GBUFS = 16                                 # gather tiles in flight
NQUEUES = 4

_cache = {}


def _build_program(C_wq):
    """Build + compile the SPMD program. C_wq: [N_WIN][N_QUAD] chunks per cell
    (identical across cores). Returns compiled Bacc."""
    dt = mybir.dt
    C_wq = [list(map(int, row)) for row in C_wq]
    S = sum(sum(row) for row in C_wq)          # total 128-edge slots
    CMAX = max(max(row) for row in C_wq)

    nc = bacc.Bacc("TRN2", target_bir_lowering=False, debug=False,
                   enable_asserts=False, num_devices=NCORES,
                   num_swdge_queues=NQUEUES)

    x_d = nc.dram_tensor("x", [N_NODES, F], dt.float32, kind="ExternalInput")
    w_d = nc.dram_tensor("w", [F, F], dt.float32, kind="ExternalInput")
    bias_d = nc.dram_tensor("bias", [F, 1], dt.float32, kind="ExternalInput")
    iota_d = nc.dram_tensor("iota", [P, P], dt.float32, kind="ExternalInput")
    gidx_d = nc.dram_tensor("gidx", [P, S * 8], dt.int16, kind="ExternalInput")
    rowl_d = nc.dram_tensor("rowl", [P, S], dt.float32, kind="ExternalInput")
    val_d = nc.dram_tensor("val", [P, S], dt.float32, kind="ExternalInput")
    yt_d = nc.dram_tensor("yt", [F, N_WIN * P], dt.float32, kind="ExternalOutput")

    with tile.TileContext(nc) as tc:
        with (
            tc.tile_pool(name="cst", bufs=1) as cst,
            tc.tile_pool(name="sbg", bufs=GBUFS) as sbg,
            tc.tile_pool(name="sbo", bufs=3) as sbo,
            tc.tile_pool(name="sby", bufs=4) as sby,
            tc.tile_pool(name="psw", bufs=4, space="PSUM") as psw,
            tc.tile_pool(name="psz", bufs=3, space="PSUM") as psz,
        ):
            iota_t = cst.tile([P, P], dt.float32)
            nc.sync.dma_start(iota_t[:], iota_d[:])
            w_t = cst.tile([F, F], dt.float32r)
            nc.sync.dma_start(w_t[:], w_d[:].bitcast(dt.float32r))
            bias_t = cst.tile([F, 1], dt.float32)
            nc.sync.dma_start(bias_t[:], bias_d[:])
            gidx_t = cst.tile([P, S * 8], dt.int16)
            nc.sync.dma_start(gidx_t[:], gidx_d[:])
            rowl_t = cst.tile([P, S], dt.float32)
            nc.sync.dma_start(rowl_t[:], rowl_d[:])
            val_t = cst.tile([P, S], dt.float32)
            nc.sync.dma_start(val_t[:], val_d[:])

            s = 0       # global slot cursor; gather (w,q) covers slots [s, s+C)
            qn = 0
            for w in range(N_WIN):
                g = []
                w_slot0 = s
                for q in range(N_QUAD):
                    C = C_wq[w][q]
                    gq = sbg.tile([P, CMAX, F], dt.float32r, tag="g")
                    if qn < GBUFS:
                        # zero each pool slot once so skipped (-1) pad indices
                        # leave finite data for the 0-weighted matmul lanes
                        nc.vector.memset(gq[:].bitcast(dt.float32), 0.0)
                    nc.gpsimd.dma_gather(
                        out_ap=gq[:, :C, :],
                        in_ap=x_d[q * QUAD : (q + 1) * QUAD, :].bitcast(dt.float32r),
                        idxs_ap=gidx_t[:, s * 8 : (s + C) * 8],
                        num_idxs=C * P,
                        num_idxs_reg=C * P,
                        elem_size=F,
                        queue_num=qn % NQUEUES,
                    )
                    qn += 1
                    g.append(gq)
                    s += C
                nslots = s - w_slot0
                # batched val-scaled one-hot for the whole window:
                # oh[p, j, r] = (iota[r] == rowl[p, s0+j]) * val[p, s0+j]
                oh = sbo.tile([P, 4 * CMAX, P], dt.float32r, tag="oh")
                rowl_b = rowl_t[:, w_slot0 : w_slot0 + nslots, None] \
                    .to_broadcast([P, nslots, P])
                val_b = val_t[:, w_slot0 : w_slot0 + nslots, None] \
                    .to_broadcast([P, nslots, P])
                iota_b = iota_t[:, None, :].to_broadcast([P, nslots, P])
                nc.vector.tensor_tensor(out=oh[:, :nslots, :], in0=rowl_b,
                                        in1=iota_b, op=mybir.AluOpType.is_equal)
                nc.vector.tensor_tensor(out=oh[:, :nslots, :],
                                        in0=oh[:, :nslots, :], in1=val_b,
                                        op=mybir.AluOpType.mult)
                pw = psw.tile([F, P], dt.float32, tag="pw")
                j = 0
                for q in range(N_QUAD):
                    for k in range(C_wq[w][q]):
                        nc.tensor.matmul(
                            pw[:], lhsT=g[q][:, k, :], rhs=oh[:, j, :],
                            start=(j == 0), stop=(j == nslots - 1),
                        )
                        j += 1
                at = sby.tile([F, P], dt.float32r, tag="at")
                nc.scalar.activation(at[:], pw[:], mybir.ActivationFunctionType.Copy)
                z = psz.tile([F, P], dt.float32, tag="z")
                nc.tensor.matmul(z[:], lhsT=w_t[:], rhs=at[:], start=True, stop=True)
                ys = sby.tile([F, P], dt.float32, tag="ys")
                nc.scalar.activation(ys[:], z[:],
                                     mybir.ActivationFunctionType.Sigmoid,
                                     bias=bias_t[:, 0:1])
                nc.sync.dma_start(yt_d[:, w * P : (w + 1) * P], ys[:])

    nc.compile()
    return nc


def _preprocess(X, edge_row, edge_col, edge_val, weight, bias):
    edge_row = np.asarray(edge_row, dtype=np.int64)
    edge_col = np.asarray(edge_col, dtype=np.int64)
    edge_val = np.asarray(edge_val, dtype=np.float32)

    if not np.all(edge_row[:-1] <= edge_row[1:]):
        o = np.argsort(edge_row, kind="stable")
        edge_row, edge_col, edge_val = edge_row[o], edge_col[o], edge_val[o]

    core = edge_row // ROWS_PER_CORE
    rl = edge_row % ROWS_PER_CORE
    wcore = rl // P
    rowl_v = (rl - wcore * P).astype(np.float32)
    quad = edge_col // QUAD
    colq = (edge_col % QUAD).astype(np.int16)

    key = ((core * N_WIN + wcore) * N_QUAD + quad).astype(np.int64)
    order = np.argsort(key, kind="stable")
    key_s = key[order]
    ncells = NCORES * N_WIN * N_QUAD
    counts = np.bincount(key_s, minlength=ncells).reshape(NCORES, N_WIN, N_QUAD)

    # chunks per (window, quadrant): max over cores so the program is SPMD-safe
    C_wq = -(-counts.max(axis=0) // P)           # [N_WIN, N_QUAD]
    C_wq = np.maximum(C_wq, 1)
    cell_cap = C_wq * P                          # idx per cell
    # flat slot space: cells in (w, q) order, each C_wq[w,q]*128 wide
    cell_off = np.zeros((N_WIN, N_QUAD), dtype=np.int64)
    flat = cell_cap.reshape(-1).cumsum()
    cell_off.reshape(-1)[1:] = flat[:-1]
    S = int(flat[-1]) // P

    starts = np.zeros(ncells, dtype=np.int64)
    starts[1:] = np.cumsum(counts.reshape(-1))[:-1]
    rank = np.arange(len(key_s), dtype=np.int64) - starts[key_s]

    core_s = core[order]
    w_s = wcore[order]
    q_s = quad[order]
    pos = cell_off[w_s, q_s] + rank              # flat position within core

    rowl_flat = np.full((NCORES, S * P), -1.0, dtype=np.float32)
    val_flat = np.zeros((NCORES, S * P), dtype=np.float32)
    # pad indices are -1: the gather's descriptor generator skips trailing
    # negative indices, so padding costs no DMA descriptors
    import os as _os
    _padval = -1 if _os.environ.get("PADNEG", "0") == "1" else 0
    gidx_flat = np.full((NCORES, S * P), _padval, dtype=np.int16)
    rowl_flat[core_s, pos] = rowl_v[order]
    val_flat[core_s, pos] = edge_val[order]
    gidx_flat[core_s, pos] = colq[order]

    rowl_m = np.ascontiguousarray(rowl_flat.reshape(NCORES, S, P).transpose(0, 2, 1))
    val_m = np.ascontiguousarray(val_flat.reshape(NCORES, S, P).transpose(0, 2, 1))
    # wrapped gather-index layout: within each slot-region, idx j -> partition
    # j%16 (replicated across the 8 16-partition groups), free column j//16.
    # Gathers cover whole cells = contiguous slot ranges, and slot size (128)
    # is a multiple of the 16-wrap, so wrapping slot-by-slot is equivalent.
    gw = gidx_flat.reshape(NCORES, S, 8, 16).transpose(0, 1, 3, 2)  # [NC,S,16,8]
    gw = np.tile(gw, (1, 1, 8, 1))                                  # [NC,S,128,8]
    gidx_m = np.ascontiguousarray(gw.transpose(0, 2, 1, 3).reshape(NCORES, P, S * 8))

    iota = np.tile(np.arange(P, dtype=np.float32)[None, :], (P, 1))
    shared = {
        "x": np.ascontiguousarray(np.asarray(X, dtype=np.float32)),
        "w": np.ascontiguousarray(np.asarray(weight, dtype=np.float32)),
        "bias": np.ascontiguousarray(np.asarray(bias, dtype=np.float32)[:, None]),
        "iota": iota,
    }
    per_core = [
        {"gidx": np.ascontiguousarray(gidx_m[c]),
         "rowl": np.ascontiguousarray(rowl_m[c]),
         "val": np.ascontiguousarray(val_m[c])}
        for c in range(NCORES)
    ]
    return C_wq, shared, per_core


def kernel(X, edge_row, edge_col, edge_val, weight, bias):
    C_wq, shared, per_core = _preprocess(X, edge_row, edge_col, edge_val,
                                         weight, bias)
    ckey = C_wq.tobytes()
    if ckey not in _cache:
        _cache[ckey] = _build_program(C_wq)
    nc = _cache[ckey]

    in_maps = [dict(shared, **per_core[c]) for c in range(NCORES)]
    res = bass_utils.run_bass_kernel_spmd(nc, in_maps, core_ids=list(range(NCORES)))

    out = np.empty((N_NODES, F), dtype=np.float32)
    for c in range(NCORES):
        out[c * ROWS_PER_CORE : (c + 1) * ROWS_PER_CORE] = \
            res.results[c]["yt"].T[:ROWS_PER_CORE]
    return out
